# revision 3
# baseline (speedup 1.0000x reference)
"""CapsuleLayer dynamic-routing kernel for 8 TRN2 NeuronCores.

Strategy: shard R(=8192) across the 8 cores (R_local=1024). Per-core x/W
shards are then small enough (~7MB bf16) to be SBUF-resident, so the 671MB
u_hat intermediate never touches HBM. The routing recurrence is restructured
using linearity of a_ij in v_j:  b_ij(t) = u_hat . (v_0+...+v_{t-1}),
so each iteration is one fused pass over r-tiles:
  u_tile (PE matmuls, K=i=8) -> b=u.V (DVE) -> softmax (ACT exp + DVE)
  -> s_partial accumulation (DVE).
Cross-core: s_j partials ([128,160] fp32, 80KB) AllReduced after passes 0/1;
pass-2 partials are summed + squashed on host.

Host-side prep swizzles x/W into the 32-aligned padded layout required for
K=8 PE matmuls (partition = (r%4)*32 + i) and converts to bf16.
"""
import numpy as np
import ml_dtypes
from contextlib import ExitStack

import concourse.bass as bass
import concourse.bacc as bacc
import concourse.tile as tile
from concourse import mybir
from concourse.bass_utils import run_bass_kernel_spmd

B, C, R, I, O = 128, 10, 8192, 8, 16
NCORES = 8
RL = R // NCORES          # 1024 r's per core
RQ = RL // 4              # 256
RT = 8                    # r's per iteration tile
NTILES = RL // RT         # 128
F32 = mybir.dt.float32
BF16 = mybir.dt.bfloat16
AX = mybir.AxisListType.X


def _bc(ap, dims):
    """Build a broadcast/permuted view of an AP. dims: list of entries that are
    either an int index into ap.ap (reuse that dim) or a tuple (0, count) for a
    broadcast dim."""
    new = []
    for d in dims:
        if isinstance(d, tuple):
            new.append([d[0], d[1]])
        else:
            new.append(ap.ap[d])
    return bass.AP(tensor=ap.tensor, offset=ap.offset, ap=new)


def _squash_emit(nc, pool, s_ap, v_out, bias_eps):
    """v_out[:, c, o] = squash(s_ap[:, c, o]) over o. All [128, C, O] f32."""
    m2 = pool.tile([B, C, O], F32, tag="sq_m2")
    nc.vector.tensor_mul(m2[:], s_ap, s_ap)
    sq = pool.tile([B, C], F32, tag="sq_sq")
    nc.vector.reduce_sum(out=sq[:], in_=m2[:], axis=AX)
    rt_ = pool.tile([B, C], F32, tag="sq_rt")
    nc.scalar.activation(rt_[:], sq[:], mybir.ActivationFunctionType.Sqrt,
                         bias=bias_eps[:], scale=1.0)
    d1 = pool.tile([B, C], F32, tag="sq_d1")
    nc.vector.tensor_scalar_add(d1[:], sq[:], 1.0)
    den = pool.tile([B, C], F32, tag="sq_den")
    nc.vector.tensor_mul(den[:], d1[:], rt_[:])
    rec = pool.tile([B, C], F32, tag="sq_rec")
    nc.vector.reciprocal(rec[:], den[:])
    scale = pool.tile([B, C], F32, tag="sq_scale")
    nc.vector.tensor_mul(scale[:], sq[:], rec[:])
    # v = s * scale  (scale broadcast over o)
    nc.vector.tensor_mul(v_out, s_ap, _bc(scale, [0, 1, (0, O)]))


def build_nc():
    nc = bacc.Bacc(None, num_devices=NCORES)
    xq_d = nc.declare_dram_parameter("xq", [128, RQ * B], BF16, isOutput=False)
    wq_d = nc.declare_dram_parameter("wq", [128, RQ * C * O], BF16, isOutput=False)
    out_d = nc.declare_dram_parameter("s2", [B, C * O], F32, isOutput=True)

    with ExitStack() as ctx:
        tc = ctx.enter_context(tile.TileContext(nc))
        consts = ctx.enter_context(tc.tile_pool(name="consts", bufs=1))
        psum = ctx.enter_context(tc.tile_pool(name="psum", bufs=2, space="PSUM"))
        psum_s = ctx.enter_context(tc.tile_pool(name="psum_s", bufs=1, space="PSUM"))
        work = ctx.enter_context(tc.tile_pool(name="work", bufs=4))
        acc = ctx.enter_context(tc.tile_pool(name="acc", bufs=1))
        dram = ctx.enter_context(tc.tile_pool(name="dram", bufs=1, space="DRAM"))

        xq = consts.tile([128, RQ, B], BF16)
        nc.sync.dma_start(out=xq[:], in_=xq_d[:].rearrange("p (q b) -> p q b", b=B))
        wq = consts.tile([128, RQ, C * O], BF16)
        nc.sync.dma_start(out=wq[:], in_=wq_d[:].rearrange("p (q f) -> p q f", f=C * O))

        bias_eps = acc.tile([B, 1], F32)
        nc.vector.memset(bias_eps[:], 1e-8)
        bias_zero = acc.tile([B, 1], F32)
        nc.vector.memset(bias_zero[:], 0.0)

        V = acc.tile([B, C, O], F32)      # running sum of v_t
        sfull = acc.tile([B, C, O], F32)  # AllReduced s_j

        # ---------------- pass 0: c uniform -> s0 = 0.1 * sum_r u_r ----------
        # One psum accumulator per m-residue (mms in an accumulation group
        # must share a tile_position / psum bank).
        s_acc0 = acc.tile([B, C, O], F32)
        for m in range(4):
            s0t = psum.tile([B, RT, 256], F32, tag="u_ps")
            s0m = s0t[:, 0, :C * O]
            for rq in range(RQ):
                nc.tensor.matmul(
                    s0m, xq[m * 32:m * 32 + 8, rq], wq[m * 32:m * 32 + 8, rq],
                    start=(rq == 0), stop=(rq == RQ - 1),
                    tile_position=(m * 32, 0))
            if m == 0:
                nc.vector.tensor_scalar_mul(
                    s_acc0[:].rearrange("b c o -> b (c o)"), s0m, 1.0 / C)
            else:
                nc.vector.scalar_tensor_tensor(
                    out=s_acc0[:].rearrange("b c o -> b (c o)"), in0=s0m,
                    scalar=1.0 / C, in1=s_acc0[:].rearrange("b c o -> b (c o)"),
                    op0=mybir.AluOpType.mult, op1=mybir.AluOpType.add)

        # helper: AllReduce src -> sfull
        def allreduce(idx, src):
            ar_in = dram.tile([B, C * O], F32, tag=f"ar_in{idx}")
            ar_out = dram.tile([B, C * O], F32, tag=f"ar_out{idx}",
                               addr_space="Shared")
            nc.gpsimd.dma_start(out=ar_in[:],
                                in_=src[:].rearrange("b c o -> b (c o)"))
            nc.gpsimd.collective_compute(
                "AllReduce", mybir.AluOpType.add,
                replica_groups=[list(range(NCORES))],
                ins=[ar_in[:].opt()], outs=[ar_out[:].opt()])
            nc.gpsimd.dma_start(out=sfull[:].rearrange("b c o -> b (c o)"),
                                in_=ar_out[:])

        allreduce(0, s_acc0)
        _squash_emit(nc, work, sfull[:], V[:], bias_eps)  # V = v0

        # ---------------- routing passes 1 and 2 -----------------------------
        for it in (1, 2):
            s_acc = acc.tile([B, C, O], F32, tag=f"s_acc{it}")
            V_exp = acc.tile([B, RT, C, O], BF16, tag=f"V_exp{it}")
            nc.vector.tensor_copy(V_exp[:], _bc(V[:], [0, (0, RT), 1, 2]))
            for ti in range(NTILES):
                m, q = ti // 32, ti % 32
                u_ps = psum.tile([B, RT, 256], F32, tag="u_ps")
                for j in range(RT):
                    rq = RT * q + j       # this tile covers r = 4*rq + m
                    nc.tensor.matmul(
                        u_ps[:, j, :C * O], xq[m * 32:m * 32 + 8, rq],
                        wq[m * 32:m * 32 + 8, rq], start=True, stop=True,
                        tile_position=(m * 32, 0))
                u_v = u_ps[:, :, :C * O].rearrange("b r (c o) -> b r c o", o=O)
                u_sb = work.tile([B, RT, C, O], BF16, tag="u_sb")
                nc.scalar.copy(u_sb[:], u_v)
                # b = sum_o u*V   (t = u*V broadcast over r; then reduce o)
                t = work.tile([B, RT, C, O], BF16, tag="t")
                nc.vector.tensor_mul(t[:], u_sb[:], V_exp[:])
                btile = work.tile([B, RT, C], F32, tag="btile")
                nc.vector.reduce_sum(out=btile[:], in_=t[:], axis=AX)
                # softmax over c (no max-subtraction; |b| is small)
                e = work.tile([B, RT, C], BF16, tag="e")
                nc.scalar.activation(e[:], btile[:],
                                     mybir.ActivationFunctionType.Exp,
                                     bias=bias_zero[:], scale=1.0)
                ssum = work.tile([B, RT], F32, tag="ssum")
                nc.vector.reduce_sum(out=ssum[:], in_=e[:], axis=AX)
                nrec = work.tile([B, RT], F32, tag="nrec")
                nc.vector.reciprocal(nrec[:], ssum[:])
                w = work.tile([B, RT, C], BF16, tag="w")
                nc.vector.tensor_mul(w[:], e[:], _bc(nrec, [0, 1, (0, C)]))
                # s += sum_r w*u
                w_exp = work.tile([B, RT, C, O], BF16, tag="w_exp")
                nc.scalar.copy(w_exp[:], _bc(w, [0, 1, 2, (0, O)]))
                t2 = work.tile([B, RT, C, O], BF16, tag="t2")
                nc.vector.tensor_mul(t2[:], u_sb[:], w_exp[:])
                if ti == 0:
                    nc.vector.reduce_sum(out=s_acc[:],
                                         in_=_bc(t2, [0, 2, 3, 1]), axis=AX)
                else:
                    stmp = work.tile([B, C, O], F32, tag="stmp")
                    nc.vector.reduce_sum(out=stmp[:], in_=_bc(t2, [0, 2, 3, 1]),
                                         axis=AX)
                    nc.vector.tensor_add(s_acc[:], s_acc[:], stmp[:])
            if it == 1:
                allreduce(1, s_acc)
                v1 = work.tile([B, C, O], F32, tag="v1")
                _squash_emit(nc, work, sfull[:], v1[:], bias_eps)
                nc.vector.tensor_add(V[:], V[:], v1[:])
            else:
                nc.gpsimd.dma_start(out=out_d[:],
                                    in_=s_acc[:].rearrange("b c o -> b (c o)"))
    nc.compile()
    return nc


def _prep_shards(x, w):
    """Swizzle per-core shards into the padded K=8 matmul layout (bf16)."""
    maps = []
    for core in range(NCORES):
        r0 = core * RL
        xs = x[:, r0:r0 + RL, :]                                # [B, RL, I]
        xr = xs.reshape(B, RQ, 4, I).transpose(2, 3, 1, 0)      # [4, I, RQ, B]
        xq = np.zeros((4, 32, RQ, B), dtype=ml_dtypes.bfloat16)
        xq[:, :I] = xr
        ws = w[:, r0:r0 + RL]                                   # [C, RL, I, O]
        wr = ws.reshape(C, RQ, 4, I, O).transpose(2, 3, 1, 0, 4)  # [4,I,RQ,C,O]
        wq = np.zeros((4, 32, RQ, C, O), dtype=ml_dtypes.bfloat16)
        wq[:, :I] = wr
        maps.append({"xq": xq.reshape(128, RQ * B),
                     "wq": wq.reshape(128, RQ * C * O)})
    return maps


_NC_CACHE = {}


def _postprocess(results):
    """results: list of per-core output dicts -> full [B, C, O] output."""
    s2 = np.zeros((B, C * O), dtype=np.float32)
    for i in range(NCORES):
        s2 += np.asarray(results[i]["s2"], dtype=np.float32)
    s2 = s2.reshape(B, C, O)
    sq = np.sum(s2 * s2, axis=-1, keepdims=True)
    v = (sq / (1.0 + sq)) * s2 / np.sqrt(sq + 1e-8)
    return v.astype(np.float32)


def kernel(x, route_weights, _trace=False):
    x = np.asarray(x, dtype=np.float32)
    w = np.asarray(route_weights, dtype=np.float32)
    in_maps = _prep_shards(x, w)
    if "nc" not in _NC_CACHE:
        _NC_CACHE["nc"] = build_nc()
    nc = _NC_CACHE["nc"]
    kw = {}
    if _trace:
        kw = dict(trace=True)
    try:
        res = run_bass_kernel_spmd(nc, in_maps, core_ids=list(range(NCORES)), **kw)
    except ModuleNotFoundError:
        res = run_bass_kernel_spmd(nc, in_maps, core_ids=list(range(NCORES)))
    if _trace:
        kernel._last_exec_ns = getattr(res, "exec_time_ns", None)
    return _postprocess(res.results)



# revision 7
# speedup vs baseline: 1.1284x; 1.1284x over previous
"""CapsuleLayer dynamic-routing kernel for 8 TRN2 NeuronCores.

Strategy: shard R(=8192) across the 8 cores (R_local=1024). Per-core x/W
shards are then small enough (~7MB bf16) to be SBUF-resident, so the 671MB
u_hat intermediate never touches HBM. The routing recurrence is restructured
using linearity of a_ij in v_j:  b_ij(t) = u_hat . (v_0+...+v_{t-1}),
so each iteration is one fused pass over r-tiles:
  u_tile (PE matmuls, K=i=8) -> b=u.V (DVE) -> softmax (ACT exp + DVE)
  -> s_partial accumulation (DVE).
Cross-core: s_j partials ([128,160] fp32, 80KB) AllReduced after passes 0/1;
pass-2 partials are summed + squashed on host.

Host-side prep swizzles x/W into the 32-aligned padded layout required for
K=8 PE matmuls (partition = (r%4)*32 + i) and converts to bf16.
"""
import numpy as np
import ml_dtypes
from contextlib import ExitStack

import concourse.bass as bass
import concourse.bacc as bacc
import concourse.tile as tile
from concourse import mybir
from concourse.bass_utils import run_bass_kernel_spmd

B, C, R, I, O = 128, 10, 8192, 8, 16
NCORES = 8
RL = R // NCORES          # 1024 r's per core
RQ = RL // 4              # 256
RT = 8                    # r's per iteration tile
NTILES = RL // RT         # 128
F32 = mybir.dt.float32
BF16 = mybir.dt.bfloat16
AX = mybir.AxisListType.X


def _bc(ap, dims):
    """Build a broadcast/permuted view of an AP. dims: list of entries that are
    either an int index into ap.ap (reuse that dim) or a tuple (0, count) for a
    broadcast dim."""
    new = []
    for d in dims:
        if isinstance(d, tuple):
            new.append([d[0], d[1]])
        else:
            new.append(ap.ap[d])
    return bass.AP(tensor=ap.tensor, offset=ap.offset, ap=new)


def _squash_emit(nc, pool, s_ap, v_out, bias_eps):
    """v_out[:, c, o] = squash(s_ap[:, c, o]) over o. All [128, C, O] f32."""
    m2 = pool.tile([B, C, O], F32, tag="sq_m2")
    nc.vector.tensor_mul(m2[:], s_ap, s_ap)
    sq = pool.tile([B, C], F32, tag="sq_sq")
    nc.vector.reduce_sum(out=sq[:], in_=m2[:], axis=AX)
    rt_ = pool.tile([B, C], F32, tag="sq_rt")
    nc.scalar.activation(rt_[:], sq[:], mybir.ActivationFunctionType.Sqrt,
                         bias=bias_eps[:], scale=1.0)
    d1 = pool.tile([B, C], F32, tag="sq_d1")
    nc.vector.tensor_scalar_add(d1[:], sq[:], 1.0)
    den = pool.tile([B, C], F32, tag="sq_den")
    nc.vector.tensor_mul(den[:], d1[:], rt_[:])
    rec = pool.tile([B, C], F32, tag="sq_rec")
    nc.vector.reciprocal(rec[:], den[:])
    scale = pool.tile([B, C], F32, tag="sq_scale")
    nc.vector.tensor_mul(scale[:], sq[:], rec[:])
    # v = s * scale  (scale broadcast over o)
    nc.vector.tensor_mul(v_out, s_ap, _bc(scale, [0, 1, (0, O)]))


def build_nc():
    nc = bacc.Bacc(None, num_devices=NCORES)
    # Dense (unpadded) DRAM inputs: [ (m,i)=32, ... ]. They are scattered into
    # the 32-aligned SBUF layout by DMA placement; SBUF rows 8..31 of each
    # 32-row group are never read (matmuls use K=8 slices).
    xq_d = nc.declare_dram_parameter("xq", [32, RQ * B], BF16, isOutput=False)
    wq_d = nc.declare_dram_parameter("wq", [32, RQ * C * O], BF16, isOutput=False)
    out_d = nc.declare_dram_parameter("s2", [B, C * O], F32, isOutput=True)

    with ExitStack() as ctx:
        tc = ctx.enter_context(tile.TileContext(nc))
        consts = ctx.enter_context(tc.tile_pool(name="consts", bufs=1))
        psum = ctx.enter_context(tc.tile_pool(name="psum", bufs=2, space="PSUM"))
        psum_s = ctx.enter_context(tc.tile_pool(name="psum_s", bufs=1, space="PSUM"))
        work = ctx.enter_context(tc.tile_pool(name="work", bufs=4))
        acc = ctx.enter_context(tc.tile_pool(name="acc", bufs=1))
        dram = ctx.enter_context(tc.tile_pool(name="dram", bufs=1, space="DRAM"))

        xq = consts.tile([128, RQ, B], BF16)
        wq = consts.tile([128, RQ, C * O], BF16)
        for m in range(4):
            nc.sync.dma_start(
                out=xq[:][m * 32:m * 32 + I],
                in_=xq_d[:].rearrange("p (q b) -> p q b", b=B)[m * I:(m + 1) * I])
            nc.sync.dma_start(
                out=wq[:][m * 32:m * 32 + I],
                in_=wq_d[:].rearrange("p (q f) -> p q f", f=C * O)[m * I:(m + 1) * I])

        bias_eps = acc.tile([B, 1], F32)
        nc.vector.memset(bias_eps[:], 1e-8)
        bias_zero = acc.tile([B, 1], F32)
        nc.vector.memset(bias_zero[:], 0.0)

        V = acc.tile([B, C, O], F32)      # running sum of v_t
        sfull = acc.tile([B, C, O], F32)  # AllReduced s_j

        # ---------------- pass 0: c uniform -> s0 = 0.1 * sum_r u_r ----------
        # One psum accumulator per m-residue (mms in an accumulation group
        # must share a tile_position / psum bank).
        s_acc0 = acc.tile([B, C, O], F32)
        for m in range(4):
            s0t = psum.tile([B, RT, 256], F32, tag="u_ps")
            s0m = s0t[:, 0, :C * O]
            for rq in range(RQ):
                nc.tensor.matmul(
                    s0m, xq[m * 32:m * 32 + 8, rq], wq[m * 32:m * 32 + 8, rq],
                    start=(rq == 0), stop=(rq == RQ - 1),
                    tile_position=(m * 32, 0))
            if m == 0:
                nc.vector.tensor_scalar_mul(
                    s_acc0[:].rearrange("b c o -> b (c o)"), s0m, 1.0 / C)
            else:
                nc.vector.scalar_tensor_tensor(
                    out=s_acc0[:].rearrange("b c o -> b (c o)"), in0=s0m,
                    scalar=1.0 / C, in1=s_acc0[:].rearrange("b c o -> b (c o)"),
                    op0=mybir.AluOpType.mult, op1=mybir.AluOpType.add)

        # helper: AllReduce src -> sfull
        def allreduce(idx, src):
            ar_in = dram.tile([B, C * O], F32, tag=f"ar_in{idx}")
            ar_out = dram.tile([B, C * O], F32, tag=f"ar_out{idx}",
                               addr_space="Shared")
            nc.gpsimd.dma_start(out=ar_in[:],
                                in_=src[:].rearrange("b c o -> b (c o)"))
            nc.gpsimd.collective_compute(
                "AllReduce", mybir.AluOpType.add,
                replica_groups=[list(range(NCORES))],
                ins=[ar_in[:].opt()], outs=[ar_out[:].opt()])
            nc.gpsimd.dma_start(out=sfull[:].rearrange("b c o -> b (c o)"),
                                in_=ar_out[:])

        allreduce(0, s_acc0)
        _squash_emit(nc, work, sfull[:], V[:], bias_eps)  # V = v0

        # ---------------- routing passes 1 and 2 -----------------------------
        for it in (1, 2):
            s_acc = acc.tile([B, C, O], F32, tag=f"s_acc{it}")
            V_exp = acc.tile([B, RT, C, O], BF16, tag=f"V_exp{it}")
            nc.vector.tensor_copy(V_exp[:], _bc(V[:], [0, (0, RT), 1, 2]))
            for ti in range(NTILES):
                m, q = ti // 32, ti % 32
                u_ps = psum.tile([B, RT, 256], F32, tag="u_ps")
                for j in range(RT):
                    rq = RT * q + j       # this tile covers r = 4*rq + m
                    nc.tensor.matmul(
                        u_ps[:, j, :C * O], xq[m * 32:m * 32 + 8, rq],
                        wq[m * 32:m * 32 + 8, rq], start=True, stop=True,
                        tile_position=(m * 32, 0))
                u_v = u_ps[:, :, :C * O].rearrange("b r (c o) -> b r c o", o=O)
                u_sb = work.tile([B, RT, C, O], BF16, tag="u_sb")
                nc.scalar.copy(u_sb[:], u_v)
                # b = sum_o u*V   (t = u*V broadcast over r; then reduce o)
                t = work.tile([B, RT, C, O], BF16, tag="t")
                nc.vector.tensor_mul(t[:], u_sb[:], V_exp[:])
                btile = work.tile([B, RT, C], F32, tag="btile")
                nc.vector.reduce_sum(out=btile[:], in_=t[:], axis=AX)
                # softmax over c (no max-subtraction; |b| is small)
                e = work.tile([B, RT, C], BF16, tag="e")
                nc.scalar.activation(e[:], btile[:],
                                     mybir.ActivationFunctionType.Exp,
                                     bias=bias_zero[:], scale=1.0)
                ssum = work.tile([B, RT], F32, tag="ssum")
                nc.vector.reduce_sum(out=ssum[:], in_=e[:], axis=AX)
                nrec = work.tile([B, RT], F32, tag="nrec")
                nc.vector.reciprocal(nrec[:], ssum[:])
                w = work.tile([B, RT, C], BF16, tag="w")
                nc.vector.tensor_mul(w[:], e[:], _bc(nrec, [0, 1, (0, C)]))
                # s += sum_r w*u
                w_exp = work.tile([B, RT, C, O], BF16, tag="w_exp")
                nc.scalar.copy(w_exp[:], _bc(w, [0, 1, 2, (0, O)]))
                t2 = work.tile([B, RT, C, O], BF16, tag="t2")
                nc.vector.tensor_mul(t2[:], u_sb[:], w_exp[:])
                if ti == 0:
                    nc.vector.reduce_sum(out=s_acc[:],
                                         in_=_bc(t2, [0, 2, 3, 1]), axis=AX)
                else:
                    stmp = work.tile([B, C, O], F32, tag="stmp")
                    nc.vector.reduce_sum(out=stmp[:], in_=_bc(t2, [0, 2, 3, 1]),
                                         axis=AX)
                    nc.vector.tensor_add(s_acc[:], s_acc[:], stmp[:])
            if it == 1:
                allreduce(1, s_acc)
                v1 = work.tile([B, C, O], F32, tag="v1")
                _squash_emit(nc, work, sfull[:], v1[:], bias_eps)
                nc.vector.tensor_add(V[:], V[:], v1[:])
            else:
                nc.gpsimd.dma_start(out=out_d[:],
                                    in_=s_acc[:].rearrange("b c o -> b (c o)"))
    nc.compile()
    return nc


def _prep_shards(x, w):
    """Swizzle per-core shards into the dense K=8 matmul layout (bf16)."""
    maps = []
    for core in range(NCORES):
        r0 = core * RL
        xs = x[:, r0:r0 + RL, :]                                # [B, RL, I]
        xr = xs.reshape(B, RQ, 4, I).transpose(2, 3, 1, 0)      # [4, I, RQ, B]
        xq = np.ascontiguousarray(xr).astype(ml_dtypes.bfloat16)
        ws = w[:, r0:r0 + RL]                                   # [C, RL, I, O]
        wr = ws.reshape(C, RQ, 4, I, O).transpose(2, 3, 1, 0, 4)  # [4,I,RQ,C,O]
        wq = np.ascontiguousarray(wr).astype(ml_dtypes.bfloat16)
        maps.append({"xq": xq.reshape(32, RQ * B),
                     "wq": wq.reshape(32, RQ * C * O)})
    return maps


_NC_CACHE = {}


def _postprocess(results):
    """results: list of per-core output dicts -> full [B, C, O] output."""
    s2 = np.zeros((B, C * O), dtype=np.float32)
    for i in range(NCORES):
        s2 += np.asarray(results[i]["s2"], dtype=np.float32)
    s2 = s2.reshape(B, C, O)
    sq = np.sum(s2 * s2, axis=-1, keepdims=True)
    v = (sq / (1.0 + sq)) * s2 / np.sqrt(sq + 1e-8)
    return v.astype(np.float32)


def kernel(x, route_weights, _trace=False):
    x = np.asarray(x, dtype=np.float32)
    w = np.asarray(route_weights, dtype=np.float32)
    in_maps = _prep_shards(x, w)
    if "nc" not in _NC_CACHE:
        _NC_CACHE["nc"] = build_nc()
    nc = _NC_CACHE["nc"]
    kw = {}
    if _trace:
        kw = dict(trace=True)
    try:
        res = run_bass_kernel_spmd(nc, in_maps, core_ids=list(range(NCORES)), **kw)
    except ModuleNotFoundError:
        res = run_bass_kernel_spmd(nc, in_maps, core_ids=list(range(NCORES)))
    if _trace:
        kernel._last_exec_ns = getattr(res, "exec_time_ns", None)
    return _postprocess(res.results)

